# revision 15
# baseline (speedup 1.0000x reference)
"""GAT (graph attention) kernel for Trainium2, sharded across 8 NeuronCores.

Math: for each head h the reference computes
    e   = leakyrelu(src_i + tgt_j)            (slope 0.2)
    att = softmax(where(mask, e, -9e16))
    out = att_E @ ht_e + att_N @ ht_n, then mean over heads.

Key identity used here: with s = src_i + tgt_j,
    exp(leakyrelu(s)) = e^{0.2 s} * max(1, e^{0.8 s})
                      = e^{0.2 src_i} * [ max(P'_i * QT_j, T_j) ]   with
    P'_i = e^{0.8 src_i},  QT_j = e^{tgt_j},  T_j = e^{0.2 tgt_j}
and the row factor e^{0.2 src_i} cancels in the softmax ratio.  So the
unnormalised attention is u[j,i] = mask[j,i] * max(P'_i*QT_j, T_j), which is
one fused tensor_scalar (mult+max) plus one tensor_tensor (mask multiply)
per element - no per-element exp / leakyrelu / row-max passes at all.  The
softmax denominator comes for free from a ones-column appended to V in the
attention @ V matmul.

Sharding: destination rows (N dim) split across 8 cores, 256 rows each
(row-parallel attention).  Embeddings/weights replicated.  Host does only
dtype casts / transposes / slicing; all compute is on device.
"""

import os
from contextlib import ExitStack

import numpy as np

import concourse.bass as bass
import concourse.bacc as bacc
import concourse.mybir as mybir
import concourse.tile as tile
from concourse.bass_utils import run_bass_kernel_spmd
from concourse.masks import make_identity

N, E, F_IN, H, D = 2048, 4096, 256, 8, 64
NCORES = 8
R = N // NCORES          # 256 destination rows per core
RC = R // 128            # 2 row chunks of 128
NCH = N // 128           # 16 node j-chunks
ECH = E // 128           # 32 edge j-chunks
F16 = mybir.dt.float16
F32 = mybir.dt.float32

_PROGRAM = None


def _build_program() -> bass.Bass:
    nc = bacc.Bacc("TRN2", target_bir_lowering=False, debug=False)

    maskN_d = nc.dram_tensor("maskN", [N, R], F16, kind="ExternalInput")
    maskE_d = nc.dram_tensor("maskE", [E, R], F16, kind="ExternalInput")
    nodesT_d = nc.dram_tensor("nodesT", [F_IN, N], F16, kind="ExternalInput")
    edgesT_d = nc.dram_tensor("edgesT", [F_IN, E], F16, kind="ExternalInput")
    ownT_d = nc.dram_tensor("ownT", [F_IN, R], F16, kind="ExternalInput")
    WN_d = nc.dram_tensor("WN16", [F_IN, H * D], F16, kind="ExternalInput")
    WE_d = nc.dram_tensor("WE16", [F_IN, H * D], F16, kind="ExternalInput")
    WNT_d = nc.dram_tensor("WNT16", [D, H * F_IN], F16, kind="ExternalInput")
    WET_d = nc.dram_tensor("WET16", [D, H * F_IN], F16, kind="ExternalInput")
    aN_d = nc.dram_tensor("aN16", [D, 3 * H], F16, kind="ExternalInput")
    aE_d = nc.dram_tensor("aE16", [D, H], F16, kind="ExternalInput")
    out_d = nc.dram_tensor("out", [R, D], F32, kind="ExternalOutput")

    with tile.TileContext(nc) as tc, ExitStack() as ctx:
        singles = ctx.enter_context(tc.tile_pool(name="singles", bufs=1))
        work = ctx.enter_context(tc.tile_pool(name="work", bufs=6))
        small = ctx.enter_context(tc.tile_pool(name="small", bufs=4))
        psum_ht = ctx.enter_context(tc.tile_pool(name="psum_ht", bufs=3, space="PSUM"))
        psum_acc = ctx.enter_context(tc.tile_pool(name="psum_acc", bufs=3, space="PSUM"))
        psum_misc = ctx.enter_context(tc.tile_pool(name="psum_misc", bufs=2, space="PSUM"))
        dram_pool = ctx.enter_context(tc.tile_pool(name="dram", bufs=1, space="DRAM"))

        # ---- persistent SBUF arrays -------------------------------------
        maskN = singles.tile([128, NCH, R], F16, tag="maskN")
        maskE = singles.tile([128, ECH, R], F16, tag="maskE")
        nodesT = singles.tile([128, 2, N], F16, tag="nodesT")
        edgesT = singles.tile([128, 2, E], F16, tag="edgesT")
        ownT = singles.tile([128, 2, R], F16, tag="ownT")
        WN = singles.tile([128, 2, H * D], F16, tag="WN")
        WE = singles.tile([128, 2, H * D], F16, tag="WE")
        WNT = singles.tile([D, H, F_IN], F16, tag="WNT")
        WET = singles.tile([D, H, F_IN], F16, tag="WET")
        aN = singles.tile([D, 3 * H], F16, tag="aN")
        aE = singles.tile([D, H], F16, tag="aE")
        # per-j exponential vectors (j-chunk partition layout)
        e10n = singles.tile([128, NCH, 3 * H], F32, tag="e10n")  # e^{tgt} cols
        e2n = singles.tile([128, NCH, 3 * H], F32, tag="e2n")    # e^{0.2 tgt}
        e10e = singles.tile([128, ECH, H], F32, tag="e10e")
        e2e = singles.tile([128, ECH, H], F32, tag="e2e")
        # V tiles: [ht | 1] per (j-chunk, head); 66-wide for 4B alignment
        VT = singles.tile([128, NCH + ECH, H, 66], F16, tag="VT")
        # P' broadcast tiles per (head, part): [j-partition-bcast, 256 i]
        Pb = singles.tile([128, 2 * H, R], F16, tag="Pb")
        srcrows = singles.tile([3 * H, RC, 128], F16, tag="srcrows")
        identF16 = singles.tile([128, 128], F16, tag="identF16")
        identF32 = singles.tile([128, 128], F32, tag="identF32")
        acc = singles.tile([128, RC, D], F32, tag="acc")

        make_identity(nc, identF16)
        make_identity(nc, identF32)
        nc.vector.memset(VT[:, :, :, 64:66], 0.0)
        nc.vector.memset(VT[:, :, :, 64:65], 1.0)
        nc.vector.memset(acc, 0.0)

        # ---- input DMAs --------------------------------------------------
        nc.sync.dma_start(out=maskN, in_=maskN_d.rearrange("(t p) i -> p t i", p=128))
        nc.sync.dma_start(out=maskE, in_=maskE_d.rearrange("(t p) i -> p t i", p=128))
        nc.sync.dma_start(out=nodesT, in_=nodesT_d.rearrange("(c p) n -> p c n", p=128))
        nc.sync.dma_start(out=edgesT, in_=edgesT_d.rearrange("(c p) n -> p c n", p=128))
        nc.sync.dma_start(out=ownT, in_=ownT_d.rearrange("(c p) n -> p c n", p=128))
        nc.sync.dma_start(out=WN, in_=WN_d.rearrange("(c p) m -> p c m", p=128))
        nc.sync.dma_start(out=WE, in_=WE_d.rearrange("(c p) m -> p c m", p=128))
        nc.sync.dma_start(out=WNT, in_=WNT_d.rearrange("d (h k) -> d h k", h=H))
        nc.sync.dma_start(out=WET, in_=WET_d.rearrange("d (h k) -> d h k", h=H))
        nc.sync.dma_start(out=aN, in_=aN_d[:, :])
        nc.sync.dma_start(out=aE, in_=aE_d[:, :])

        # ---- attention weight vectors: wv[k, v] = sum_d W[k, hd] a[h, d] --
        wvN = singles.tile([128, 2, 3 * H], F16, tag="wvN")
        wvE = singles.tile([128, 2, H], F16, tag="wvE")
        for h in range(H):
            for kc in range(2):
                pw = psum_misc.tile([128, 3], F32, tag="pm")
                nc.tensor.matmul(
                    pw[:, :],
                    WNT[:, h, kc * 128:(kc + 1) * 128],
                    aN[:, 3 * h:3 * h + 3],
                )
                nc.scalar.activation(wvN[:, kc, 3 * h:3 * h + 3], pw[:, :],
                                     mybir.ActivationFunctionType.Copy)
                pe = psum_misc.tile([128, 1], F32, tag="pm")
                nc.tensor.matmul(
                    pe[:, :],
                    WET[:, h, kc * 128:(kc + 1) * 128],
                    aE[:, h:h + 1],
                )
                nc.scalar.activation(wvE[:, kc, h:h + 1], pe[:, :],
                                     mybir.ActivationFunctionType.Copy)

        # ---- src/tgt scores -> per-j exponentials ------------------------
        Exp = mybir.ActivationFunctionType.Exp
        for ch in range(NCH):
            ps = psum_misc.tile([128, 3 * H], F32, tag="pm")
            for kc in range(2):
                nc.tensor.matmul(
                    ps[:, :],
                    nodesT[:, kc, ch * 128:(ch + 1) * 128],
                    wvN[:, kc, :],
                    start=(kc == 0), stop=(kc == 1),
                )
            nc.scalar.activation(e10n[:, ch, :], ps[:, :], Exp, scale=1.0)
            nc.scalar.activation(e2n[:, ch, :], ps[:, :], Exp, scale=0.2)
        for ch in range(ECH):
            ps = psum_misc.tile([128, H], F32, tag="pm")
            for kc in range(2):
                nc.tensor.matmul(
                    ps[:, :],
                    edgesT[:, kc, ch * 128:(ch + 1) * 128],
                    wvE[:, kc, :],
                    start=(kc == 0), stop=(kc == 1),
                )
            nc.scalar.activation(e10e[:, ch, :], ps[:, :], Exp, scale=1.0)
            nc.scalar.activation(e2e[:, ch, :], ps[:, :], Exp, scale=0.2)

        # ---- own-row P' = e^{0.8 src} and its free-dim broadcast ---------
        e8own = small.tile([128, RC, 3 * H], F16, tag="e8own")
        for ch in range(RC):
            ps = psum_misc.tile([128, 3 * H], F32, tag="pm")
            for kc in range(2):
                nc.tensor.matmul(
                    ps[:, :],
                    ownT[:, kc, ch * 128:(ch + 1) * 128],
                    wvN[:, kc, :],
                    start=(kc == 0), stop=(kc == 1),
                )
            nc.scalar.activation(e8own[:, ch, :], ps[:, :], Exp, scale=0.8)
        for ch in range(RC):
            pt = psum_misc.tile([3 * H, 128], F16, tag="pm")
            nc.tensor.transpose(pt[:, :], e8own[:, ch, :], identF16[:, :])
            nc.scalar.activation(srcrows[:, ch, :], pt[:, :],
                                 mybir.ActivationFunctionType.Copy)
        scratch = dram_pool.tile([3 * H, R], F16)
        nc.sync.dma_start(
            out=scratch[:, :].rearrange("v (c p) -> v c p", c=RC), in_=srcrows[:, :, :]
        )
        for h in range(H):
            for part in range(2):  # 0 = N (srcN col 3h), 1 = E (srcE col 3h+2)
                v = 3 * h + 2 * part
                nc.sync.dma_start(
                    out=Pb[:, 2 * h + part, :],
                    in_=scratch[v:v + 1, :].to_broadcast((128, R)),
                )

        # ---- ht = emb @ W, stored as [ht | 1] fp16 V tiles ---------------
        for ch in range(NCH + ECH):
            ph = psum_ht.tile([128, H * D], F32, tag="ph")
            for kc in range(2):
                if ch < NCH:
                    lhsT = nodesT[:, kc, ch * 128:(ch + 1) * 128]
                    rhs = WN[:, kc, :]
                else:
                    lhsT = edgesT[:, kc, (ch - NCH) * 128:(ch - NCH + 1) * 128]
                    rhs = WE[:, kc, :]
                nc.tensor.matmul(ph[:, :], lhsT, rhs, start=(kc == 0), stop=(kc == 1))
            nc.scalar.activation(
                VT[:, ch, :, 0:64],
                ph[:, :].rearrange("p (h d) -> p h d", h=H),
                mybir.ActivationFunctionType.Copy,
            )

        # ---- main loop: u = mask * max(P'*QT, T); S += u^T-contract V ----
        for h in range(H):
            Sn = psum_acc.tile([65, R], F32, tag="Sacc")
            Se = psum_acc.tile([65, R], F32, tag="Sacc")
            for part, (njt, Spsum) in enumerate(((NCH, Sn), (ECH, Se))):
                for jt in range(njt):
                    if part == 0:
                        q10 = e10n[:, jt, 3 * h + 1:3 * h + 2]
                        q2 = e2n[:, jt, 3 * h + 1:3 * h + 2]
                        msk = maskN[:, jt, :]
                        vch = jt
                    else:
                        q10 = e10e[:, jt, h:h + 1]
                        q2 = e2e[:, jt, h:h + 1]
                        msk = maskE[:, jt, :]
                        vch = NCH + jt
                    dt_ = work.tile([128, R], F16, tag="Dt")
                    nc.vector.tensor_scalar(
                        out=dt_[:, :], in0=Pb[:, 2 * h + part, :],
                        scalar1=q10, scalar2=q2,
                        op0=mybir.AluOpType.mult, op1=mybir.AluOpType.max,
                    )
                    ut = work.tile([128, R], F16, tag="ut")
                    nc.vector.tensor_mul(ut[:, :], dt_[:, :], msk)
                    nc.tensor.matmul(
                        Spsum[:, :], VT[:, vch, h, 0:65], ut[:, :],
                        start=(jt == 0), stop=(jt == njt - 1),
                    )
            # ---- normalize + accumulate over heads -----------------------
            for part, Spsum in enumerate((Sn, Se)):
                Ssb = small.tile([65, R], F32, tag="Ssb")
                nc.scalar.activation(Ssb[:, :], Spsum[:, :],
                                     mybir.ActivationFunctionType.Copy)
                for ch in range(RC):
                    ptt = psum_misc.tile([128, 65], F32, tag="pm")
                    nc.tensor.transpose(ptt[:, :], Ssb[:, ch * 128:(ch + 1) * 128],
                                        identF32[:65, :65])
                    Tt = small.tile([128, 65], F32, tag="Tt")
                    nc.scalar.activation(Tt[:, :], ptt[:, :],
                                         mybir.ActivationFunctionType.Copy)
                    rec = small.tile([128, 1], F32, tag="rec")
                    nc.vector.reciprocal(rec[:, :], Tt[:, 64:65])
                    contrib = small.tile([128, D], F32, tag="contrib")
                    nc.vector.tensor_scalar_mul(contrib[:, :], Tt[:, 0:64], rec[:, :])
                    nc.vector.tensor_add(acc[:, ch, :], acc[:, ch, :], contrib[:, :])

        out_sb = singles.tile([128, RC, D], F32, tag="out_sb")
        nc.scalar.mul(out_sb[:, :, :], acc[:, :, :], 1.0 / H)
        nc.sync.dma_start(out=out_d.rearrange("(c p) d -> p c d", p=128), in_=out_sb)

    return nc


def _get_program() -> bass.Bass:
    global _PROGRAM
    if _PROGRAM is None:
        nc = _build_program()
        nc.finalize()
        _PROGRAM = nc
    return _PROGRAM


def _prepare_in_maps(inputs) -> list:
    nodes = np.ascontiguousarray(np.asarray(inputs["nodes_embeddings"], np.float32))
    edges = np.ascontiguousarray(np.asarray(inputs["edges_embeddings"], np.float32))
    WN = np.asarray(inputs["WN"], np.float32)
    WE = np.asarray(inputs["WE"], np.float32)
    aN = np.asarray(inputs["aN"], np.float32)
    aE = np.asarray(inputs["aE"], np.float32)
    mat_nodes = np.asarray(inputs["mat_nodes"])
    mat_edges = np.asarray(inputs["mat_edges"])

    f16 = np.float16
    nodesT16 = np.ascontiguousarray(nodes.T.astype(f16))
    edgesT16 = np.ascontiguousarray(edges.T.astype(f16))
    WN16 = WN.astype(f16)
    WE16 = WE.astype(f16)
    WNT16 = np.ascontiguousarray(
        WN.T.astype(f16).reshape(H, D, F_IN).transpose(1, 0, 2).reshape(D, H * F_IN))
    WET16 = np.ascontiguousarray(
        WE.T.astype(f16).reshape(H, D, F_IN).transpose(1, 0, 2).reshape(D, H * F_IN))
    # aN16 cols per head: [srcN, tgtN, srcE]; aE16 col per head: tgtE
    aN16 = np.empty((D, 3 * H), f16)
    aE16 = np.empty((D, H), f16)
    for h in range(H):
        aN16[:, 3 * h] = aN[h, :D, 0].astype(f16)
        aN16[:, 3 * h + 1] = aN[h, D:, 0].astype(f16)
        aN16[:, 3 * h + 2] = aE[h, :D, 0].astype(f16)
        aE16[:, h] = aE[h, D:, 0].astype(f16)
    maskN_T = np.ascontiguousarray(mat_nodes.astype(f16).T)  # [j, i_global]
    maskE_T = np.ascontiguousarray(mat_edges.astype(f16).T)

    in_maps = []
    for c in range(NCORES):
        sl = slice(c * R, (c + 1) * R)
        in_maps.append({
            "maskN": np.ascontiguousarray(maskN_T[:, sl]),
            "maskE": np.ascontiguousarray(maskE_T[:, sl]),
            "nodesT": nodesT16,
            "edgesT": edgesT16,
            "ownT": np.ascontiguousarray(nodesT16[:, sl]),
            "WN16": WN16,
            "WE16": WE16,
            "WNT16": WNT16,
            "WET16": WET16,
            "aN16": aN16,
            "aE16": aE16,
        })
    return in_maps


def kernel(**inputs) -> np.ndarray:
    in_maps = _prepare_in_maps(inputs)
    nc = _get_program()
    res = run_bass_kernel_spmd(nc, in_maps, core_ids=list(range(NCORES)))
    return np.concatenate([res.results[c]["out"] for c in range(NCORES)], axis=0)


# revision 27
# speedup vs baseline: 517.2360x; 517.2360x over previous
"""GAT (graph attention) kernel for Trainium2, sharded across 8 NeuronCores.

Math: for each head h the reference computes
    e   = leakyrelu(src_i + tgt_j)            (slope 0.2)
    att = softmax(where(mask, e, -9e16))
    out = att_E @ ht_e + att_N @ ht_n, then mean over heads.

Key identity used here: with s = src_i + tgt_j,
    exp(leakyrelu(s)) = e^{0.2 s} * max(1, e^{0.8 s})
                      = e^{0.2 src_i} * [ max(P'_i * QT_j, T_j) ]   with
    P'_i = e^{0.8 src_i},  QT_j = e^{tgt_j},  T_j = e^{0.2 tgt_j}
and the row factor e^{0.2 src_i} cancels in the softmax ratio.  So the
unnormalised attention is u[j,i] = mask[j,i] * max(P'_i*QT_j, T_j), which is
one fused tensor_scalar (mult+max) plus one tensor_tensor (mask multiply)
per element - no per-element exp / leakyrelu / row-max passes at all.  The
softmax denominator comes for free from a ones-column appended to V in the
attention @ V matmul.

Sharding: destination rows (N dim) split across 8 cores, 256 rows each
(row-parallel attention).  Embeddings/weights replicated.  Host does only
dtype casts / transposes / slicing; all compute is on device.
"""

import os
from contextlib import ExitStack

import numpy as np

import concourse.bass as bass
import concourse.bacc as bacc
import concourse.mybir as mybir
import concourse.tile as tile
from concourse.bass_utils import run_bass_kernel_spmd
from concourse.masks import make_identity

N, E, F_IN, H, D = 2048, 4096, 256, 8, 64
NCORES = 8
R = N // NCORES          # 256 destination rows per core
RC = R // 128            # 2 row chunks of 128
NCH = N // 128           # 16 node j-chunks
ECH = E // 128           # 32 edge j-chunks
F16 = mybir.dt.float16
F32 = mybir.dt.float32

_PROGRAM = None


def _build_program() -> bass.Bass:
    nc = bacc.Bacc("TRN2", target_bir_lowering=False, debug=False)

    maskN_d = nc.dram_tensor("maskN", [N, R], F16, kind="ExternalInput")
    maskE_d = nc.dram_tensor("maskE", [E, R], F16, kind="ExternalInput")
    nodesT_d = nc.dram_tensor("nodesT", [F_IN, N], F16, kind="ExternalInput")
    edgesT_d = nc.dram_tensor("edgesT", [F_IN, E], F16, kind="ExternalInput")
    ownT_d = nc.dram_tensor("ownT", [F_IN, R], F16, kind="ExternalInput")
    WN_d = nc.dram_tensor("WN16", [F_IN, H * D], F16, kind="ExternalInput")
    WE_d = nc.dram_tensor("WE16", [F_IN, H * D], F16, kind="ExternalInput")
    WNT_d = nc.dram_tensor("WNT16", [D, H * F_IN], F16, kind="ExternalInput")
    WET_d = nc.dram_tensor("WET16", [D, H * F_IN], F16, kind="ExternalInput")
    aN_d = nc.dram_tensor("aN16", [D, 3 * H], F16, kind="ExternalInput")
    aE_d = nc.dram_tensor("aE16", [D, H], F16, kind="ExternalInput")
    out_d = nc.dram_tensor("out", [R, D], F32, kind="ExternalOutput")

    with tile.TileContext(nc) as tc, ExitStack() as ctx:
        singles = ctx.enter_context(tc.tile_pool(name="singles", bufs=1))
        work = ctx.enter_context(tc.tile_pool(name="work", bufs=6))
        small = ctx.enter_context(tc.tile_pool(name="small", bufs=4))
        psum_ht = ctx.enter_context(tc.tile_pool(name="psum_ht", bufs=3, space="PSUM"))
        psum_acc = ctx.enter_context(tc.tile_pool(name="psum_acc", bufs=3, space="PSUM"))
        psum_misc = ctx.enter_context(tc.tile_pool(name="psum_misc", bufs=2, space="PSUM"))
        dram_pool = ctx.enter_context(tc.tile_pool(name="dram", bufs=1, space="DRAM"))

        # ---- persistent SBUF arrays -------------------------------------
        maskN = singles.tile([128, NCH, R], F16, tag="maskN")
        maskE = singles.tile([128, ECH, R], F16, tag="maskE")
        nodesT = singles.tile([128, 2, N], F16, tag="nodesT")
        edgesT = singles.tile([128, 2, E], F16, tag="edgesT")
        ownT = singles.tile([128, 2, R], F16, tag="ownT")
        WN = singles.tile([128, 2, H * D], F16, tag="WN")
        WE = singles.tile([128, 2, H * D], F16, tag="WE")
        WNT = singles.tile([D, H, F_IN], F16, tag="WNT")
        WET = singles.tile([D, H, F_IN], F16, tag="WET")
        aN = singles.tile([D, 3 * H], F16, tag="aN")
        aE = singles.tile([D, H], F16, tag="aE")
        # per-j exponential vectors (j-chunk partition layout)
        e10n = singles.tile([128, NCH, 3 * H], F32, tag="e10n")  # e^{tgt} cols
        e2n = singles.tile([128, NCH, 3 * H], F32, tag="e2n")    # e^{0.2 tgt}
        e10e = singles.tile([128, ECH, H], F32, tag="e10e")
        e2e = singles.tile([128, ECH, H], F32, tag="e2e")
        # V tiles: [ht | 1] per (j-chunk, head); 66-wide for 4B alignment
        VT = singles.tile([128, NCH + ECH, H, 66], F16, tag="VT")
        # P' broadcast tiles per (head, part): [j-partition-bcast, 256 i]
        Pb = singles.tile([128, H, 2, R], F16, tag="Pb")
        ptsb = singles.tile([2 * H, RC, 128], F16, tag="ptsb")
        sel16 = singles.tile([2 * H, 2 * H * 128], F16, tag="sel16")
        identF16 = singles.tile([128, 128], F16, tag="identF16")
        identF32 = singles.tile([128, 128], F32, tag="identF32")
        acc = singles.tile([128, RC, D], F32, tag="acc")

        make_identity(nc, identF16)
        make_identity(nc, identF32)
        nc.gpsimd.memset(sel16, 0.0)
        nc.gpsimd.affine_select(
            out=sel16, in_=sel16, compare_op=mybir.AluOpType.not_equal,
            fill=1.0, base=0, pattern=[[-1, 2 * H], [0, 128]],
            channel_multiplier=1,
        )
        nc.vector.memset(VT[:, :, :, 64:66], 0.0)
        nc.vector.memset(VT[:, :, :, 64:65], 1.0)
        nc.vector.memset(acc, 0.0)

        # ---- input DMAs (critical-path tensors first; masks split so the
        # first j-chunks land early) ---------------------------------------
        nc.sync.dma_start(out=aN, in_=aN_d[:, :])
        nc.sync.dma_start(out=aE, in_=aE_d[:, :])
        nc.sync.dma_start(out=WNT, in_=WNT_d.rearrange("d (h k) -> d h k", h=H))
        nc.sync.dma_start(out=WET, in_=WET_d.rearrange("d (h k) -> d h k", h=H))
        nc.sync.dma_start(out=ownT, in_=ownT_d.rearrange("(c p) n -> p c n", p=128))
        nc.sync.dma_start(out=nodesT, in_=nodesT_d.rearrange("(c p) n -> p c n", p=128))
        nc.sync.dma_start(out=WN, in_=WN_d.rearrange("(c p) m -> p c m", p=128))
        nc.sync.dma_start(out=maskN, in_=maskN_d.rearrange("(t p) i -> p t i", p=128))
        nc.sync.dma_start(out=edgesT, in_=edgesT_d.rearrange("(c p) n -> p c n", p=128))
        nc.sync.dma_start(out=WE, in_=WE_d.rearrange("(c p) m -> p c m", p=128))
        nc.sync.dma_start(out=maskE, in_=maskE_d.rearrange("(t p) i -> p t i", p=128))

        # ---- attention weight vectors: wv[k, v] = sum_d W[k, hd] a[h, d] --
        wvN = singles.tile([128, 2, 3 * H], F16, tag="wvN")
        wvE = singles.tile([128, 2, H], F16, tag="wvE")
        Copy = mybir.ActivationFunctionType.Copy
        Exp = mybir.ActivationFunctionType.Exp
        pwv = psum_misc.tile([128, 2, 3 * H], F32, tag="pm")
        pwe = psum_misc.tile([128, 2, H], F32, tag="pm")
        for kc in range(2):
            for h in range(H):
                nc.tensor.matmul(pwv[:, kc, 3 * h:3 * h + 3],
                                 WNT[:, h, kc * 128:(kc + 1) * 128],
                                 aN[:, 3 * h:3 * h + 3])
                nc.tensor.matmul(pwe[:, kc, h:h + 1],
                                 WET[:, h, kc * 128:(kc + 1) * 128],
                                 aE[:, h:h + 1])
        nc.scalar.activation(wvN[:, :, :], pwv[:, :, :], Copy)
        nc.scalar.activation(wvE[:, :, :], pwe[:, :, :], Copy)

        # ---- own-row P' = e^{0.8 src} -> free-dim broadcast tiles (first:
        # everything the main loop's first iterations depend on) -----------
        e8own = small.tile([128, RC, 3 * H], F16, tag="e8own")
        pso = psum_misc.tile([128, RC, 3 * H], F32, tag="pm")
        for ch in range(RC):
            for kc in range(2):
                nc.tensor.matmul(pso[:, ch, :],
                                 ownT[:, kc, ch * 128:(ch + 1) * 128],
                                 wvN[:, kc, :], start=(kc == 0), stop=(kc == 1))
        nc.scalar.activation(e8own[:, :, :], pso[:, :, :], Exp, scale=0.8)
        # transpose the 16 needed src columns (u = 2h+part <- col 3h+2*part)
        # to [16, i] rows at base partition 0, then broadcast each row across
        # all 128 partitions with a selector matmul (row u of sel16 block u
        # is all-ones).
        e8cols = small.tile([128, RC, 2 * H], F16, tag="e8cols")
        for ch in range(RC):
            e8sl = e8own[:, ch, :]
            cols = bass.AP(tensor=e8sl.tensor, offset=e8sl.offset,
                           ap=[e8sl.ap[0], [3, H], [2, 2]])
            nc.scalar.activation(e8cols[:, ch, :], cols, Copy)
            pt = psum_misc.tile([2 * H, 128], F16, tag="pm")
            nc.tensor.transpose(pt[:, :], e8cols[:, ch, :], identF16[:, :])
            nc.scalar.activation(ptsb[:, ch, :], pt[:, :], Copy)
        for u in range(2 * H):
            h, part = u // 2, u % 2
            pb = psum_misc.tile([128, R], F32, tag="pm")
            nc.tensor.matmul(pb[:, :], sel16[:, u * 128:(u + 1) * 128],
                             ptsb[:, :, :].rearrange("u c p -> u (c p)"))
            nc.vector.tensor_copy(Pb[:, h, part, :], pb[:, :])

        # ---- src/tgt scores -> per-j exponentials (batched exps) ---------
        psn = psum_misc.tile([128, NCH, 3 * H], F32, tag="pm")
        for ch in range(NCH):
            for kc in range(2):
                nc.tensor.matmul(psn[:, ch, :],
                                 nodesT[:, kc, ch * 128:(ch + 1) * 128],
                                 wvN[:, kc, :], start=(kc == 0), stop=(kc == 1))
        nc.scalar.activation(e10n[:, :, :], psn[:, :, :], Exp, scale=1.0)
        nc.scalar.activation(e2n[:, :, :], psn[:, :, :], Exp, scale=0.2)
        # ---- ht = emb @ W, stored as [ht | 1] fp16 V tiles ---------------
        def emit_ht(ch):
            ph = psum_ht.tile([128, H * D], F32, tag="ph")
            for kc in range(2):
                if ch < NCH:
                    lhsT = nodesT[:, kc, ch * 128:(ch + 1) * 128]
                    rhs = WN[:, kc, :]
                else:
                    lhsT = edgesT[:, kc, (ch - NCH) * 128:(ch - NCH + 1) * 128]
                    rhs = WE[:, kc, :]
                nc.tensor.matmul(ph[:, :], lhsT, rhs, start=(kc == 0), stop=(kc == 1))
            nc.scalar.activation(
                VT[:, ch, :, 0:64],
                ph[:, :].rearrange("p (h d) -> p h d", h=H),
                Copy,
            )

        for ch in range(NCH):
            emit_ht(ch)

        def emit_edges_prep():
            pse = psum_misc.tile([128, ECH, H], F32, tag="pm")
            for ch in range(ECH):
                for kc in range(2):
                    nc.tensor.matmul(pse[:, ch, :],
                                     edgesT[:, kc, ch * 128:(ch + 1) * 128],
                                     wvE[:, kc, :], start=(kc == 0), stop=(kc == 1))
            nc.scalar.activation(e10e[:, :, :], pse[:, :, :], Exp, scale=1.0)
            nc.scalar.activation(e2e[:, :, :], pse[:, :, :], Exp, scale=0.2)
            for ch in range(NCH, NCH + ECH):
                emit_ht(ch)

        # ---- main loop: u = mask * max(P'*QT, T); S += u^T-contract V ----
        # all N-parts first (node-side inputs arrive early), then all
        # E-parts - the edge-side DMAs hide under the N-part compute.
        G = 8  # j-chunks per fused mask-multiply
        for part in range(2):
            njt = NCH if part == 0 else ECH
            for h in range(H):
                if part == 0 and h == 3:
                    emit_edges_prep()
                Sp = psum_acc.tile([65, R], F32, tag="Sacc")
                for jt0 in range(0, njt, G):
                    dt_ = work.tile([128, G, R], F16, tag="Dt")
                    for g in range(G):
                        jt = jt0 + g
                        if part == 0:
                            q10 = e10n[:, jt, 3 * h + 1:3 * h + 2]
                            q2 = e2n[:, jt, 3 * h + 1:3 * h + 2]
                        else:
                            q10 = e10e[:, jt, h:h + 1]
                            q2 = e2e[:, jt, h:h + 1]
                        nc.vector.tensor_scalar(
                            out=dt_[:, g, :], in0=Pb[:, h, part, :],
                            scalar1=q10, scalar2=q2,
                            op0=mybir.AluOpType.mult, op1=mybir.AluOpType.max,
                        )
                    ut = work.tile([128, G, R], F16, tag="ut")
                    msk = maskN if part == 0 else maskE
                    nc.vector.tensor_mul(ut[:, :, :], dt_[:, :, :],
                                         msk[:, jt0:jt0 + G, :])
                    for g in range(G):
                        jt = jt0 + g
                        vch = jt if part == 0 else NCH + jt
                        nc.tensor.matmul(
                            Sp[:, :], VT[:, vch, h, 0:65], ut[:, g, :],
                            start=(jt == 0), stop=(jt == njt - 1),
                        )
                # ---- normalize + accumulate into acc -------------------
                Ssb = small.tile([65, R], F32, tag="Ssb")
                nc.scalar.activation(Ssb[:, :], Sp[:, :], Copy)
                for ch in range(RC):
                    ptt = psum_misc.tile([128, 65], F32, tag="pm")
                    nc.tensor.transpose(ptt[:, :], Ssb[:, ch * 128:(ch + 1) * 128],
                                        identF32[:65, :65])
                    Tt = small.tile([128, 65], F32, tag="Tt")
                    nc.scalar.activation(Tt[:, :], ptt[:, :], Copy)
                    rec = small.tile([128, 1], F32, tag="rec")
                    nc.vector.reciprocal(rec[:, :], Tt[:, 64:65])
                    contrib = small.tile([128, D], F32, tag="contrib")
                    nc.scalar.activation(contrib[:, :], Tt[:, 0:64], Copy,
                                         scale=rec[:, :])
                    nc.vector.tensor_add(acc[:, ch, :], acc[:, ch, :], contrib[:, :])

        out_sb = singles.tile([128, RC, D], F32, tag="out_sb")
        nc.scalar.mul(out_sb[:, :, :], acc[:, :, :], 1.0 / H)
        nc.sync.dma_start(out=out_d.rearrange("(c p) d -> p c d", p=128), in_=out_sb)

    return nc


def _get_program() -> bass.Bass:
    global _PROGRAM
    if _PROGRAM is None:
        nc = _build_program()
        nc.finalize()
        _PROGRAM = nc
    return _PROGRAM


def _prepare_in_maps(inputs) -> list:
    nodes = np.ascontiguousarray(np.asarray(inputs["nodes_embeddings"], np.float32))
    edges = np.ascontiguousarray(np.asarray(inputs["edges_embeddings"], np.float32))
    WN = np.asarray(inputs["WN"], np.float32)
    WE = np.asarray(inputs["WE"], np.float32)
    aN = np.asarray(inputs["aN"], np.float32)
    aE = np.asarray(inputs["aE"], np.float32)
    mat_nodes = np.asarray(inputs["mat_nodes"])
    mat_edges = np.asarray(inputs["mat_edges"])

    f16 = np.float16
    nodesT16 = np.ascontiguousarray(nodes.T.astype(f16))
    edgesT16 = np.ascontiguousarray(edges.T.astype(f16))
    WN16 = WN.astype(f16)
    WE16 = WE.astype(f16)
    WNT16 = np.ascontiguousarray(
        WN.T.astype(f16).reshape(H, D, F_IN).transpose(1, 0, 2).reshape(D, H * F_IN))
    WET16 = np.ascontiguousarray(
        WE.T.astype(f16).reshape(H, D, F_IN).transpose(1, 0, 2).reshape(D, H * F_IN))
    # aN16 cols per head: [srcN, tgtN, srcE]; aE16 col per head: tgtE
    aN16 = np.empty((D, 3 * H), f16)
    aE16 = np.empty((D, H), f16)
    for h in range(H):
        aN16[:, 3 * h] = aN[h, :D, 0].astype(f16)
        aN16[:, 3 * h + 1] = aN[h, D:, 0].astype(f16)
        aN16[:, 3 * h + 2] = aE[h, :D, 0].astype(f16)
        aE16[:, h] = aE[h, D:, 0].astype(f16)
    maskN_T = np.ascontiguousarray(mat_nodes.astype(f16).T)  # [j, i_global]
    maskE_T = np.ascontiguousarray(mat_edges.astype(f16).T)

    in_maps = []
    for c in range(NCORES):
        sl = slice(c * R, (c + 1) * R)
        in_maps.append({
            "maskN": np.ascontiguousarray(maskN_T[:, sl]),
            "maskE": np.ascontiguousarray(maskE_T[:, sl]),
            "nodesT": nodesT16,
            "edgesT": edgesT16,
            "ownT": np.ascontiguousarray(nodesT16[:, sl]),
            "WN16": WN16,
            "WE16": WE16,
            "WNT16": WNT16,
            "WET16": WET16,
            "aN16": aN16,
            "aE16": aE16,
        })
    return in_maps


def kernel(**inputs) -> np.ndarray:
    in_maps = _prepare_in_maps(inputs)
    nc = _get_program()
    res = run_bass_kernel_spmd(nc, in_maps, core_ids=list(range(NCORES)))
    return np.concatenate([res.results[c]["out"] for c in range(NCORES)], axis=0)


# revision 31
# speedup vs baseline: 518.0439x; 1.0016x over previous
"""GAT (graph attention) kernel for Trainium2, sharded across 8 NeuronCores.

Math: for each head h the reference computes
    e   = leakyrelu(src_i + tgt_j)            (slope 0.2)
    att = softmax(where(mask, e, -9e16))
    out = att_E @ ht_e + att_N @ ht_n, then mean over heads.

Key identity used here: with s = src_i + tgt_j,
    exp(leakyrelu(s)) = e^{0.2 s} * max(1, e^{0.8 s})
                      = e^{0.2 src_i} * [ max(P'_i * QT_j, T_j) ]   with
    P'_i = e^{0.8 src_i},  QT_j = e^{tgt_j},  T_j = e^{0.2 tgt_j}
and the row factor e^{0.2 src_i} cancels in the softmax ratio.  So the
unnormalised attention is u[j,i] = mask[j,i] * max(P'_i*QT_j, T_j), which is
one fused tensor_scalar (mult+max) plus one tensor_tensor (mask multiply)
per element - no per-element exp / leakyrelu / row-max passes at all.  The
softmax denominator comes for free from a ones-column appended to V in the
attention @ V matmul.

Sharding: destination rows (N dim) split across 8 cores, 256 rows each
(row-parallel attention).  Embeddings/weights replicated.  Host does only
dtype casts / transposes / slicing; all compute is on device.
"""

import os
from contextlib import ExitStack

import numpy as np

import concourse.bass as bass
import concourse.bacc as bacc
import concourse.mybir as mybir
import concourse.tile as tile
from concourse.bass_utils import run_bass_kernel_spmd
from concourse.masks import make_identity

N, E, F_IN, H, D = 2048, 4096, 256, 8, 64
NCORES = 8
R = N // NCORES          # 256 destination rows per core
RC = R // 128            # 2 row chunks of 128
NCH = N // 128           # 16 node j-chunks
ECH = E // 128           # 32 edge j-chunks
F16 = mybir.dt.float16
F32 = mybir.dt.float32

_PROGRAM = None


def _build_program() -> bass.Bass:
    nc = bacc.Bacc("TRN2", target_bir_lowering=False, debug=False)

    maskN_d = nc.dram_tensor("maskN", [N, R], F16, kind="ExternalInput")
    maskE_d = nc.dram_tensor("maskE", [E, R], F16, kind="ExternalInput")
    nodesT_d = nc.dram_tensor("nodesT", [F_IN, N], F16, kind="ExternalInput")
    edgesT_d = nc.dram_tensor("edgesT", [F_IN, E], F16, kind="ExternalInput")
    ownT_d = nc.dram_tensor("ownT", [F_IN, R], F16, kind="ExternalInput")
    WN_d = nc.dram_tensor("WN16", [F_IN, H * D], F16, kind="ExternalInput")
    WE_d = nc.dram_tensor("WE16", [F_IN, H * D], F16, kind="ExternalInput")
    WNT_d = nc.dram_tensor("WNT16", [D, H * F_IN], F16, kind="ExternalInput")
    WET_d = nc.dram_tensor("WET16", [D, H * F_IN], F16, kind="ExternalInput")
    aN_d = nc.dram_tensor("aN16", [D, 3 * H], F16, kind="ExternalInput")
    aE_d = nc.dram_tensor("aE16", [D, H], F16, kind="ExternalInput")
    out_d = nc.dram_tensor("out", [R, D], F32, kind="ExternalOutput")

    with tile.TileContext(nc) as tc, ExitStack() as ctx:
        singles = ctx.enter_context(tc.tile_pool(name="singles", bufs=1))
        work = ctx.enter_context(tc.tile_pool(name="work", bufs=7))
        small = ctx.enter_context(tc.tile_pool(name="small", bufs=6))
        psum_ht = ctx.enter_context(tc.tile_pool(name="psum_ht", bufs=3, space="PSUM"))
        psum_acc = ctx.enter_context(tc.tile_pool(name="psum_acc", bufs=3, space="PSUM"))
        psum_misc = ctx.enter_context(tc.tile_pool(name="psum_misc", bufs=2, space="PSUM"))
        dram_pool = ctx.enter_context(tc.tile_pool(name="dram", bufs=1, space="DRAM"))

        # ---- persistent SBUF arrays -------------------------------------
        maskN = singles.tile([128, NCH, R], F16, tag="maskN")
        maskE = singles.tile([128, ECH, R], F16, tag="maskE")
        nodesT = singles.tile([128, 2, N], F16, tag="nodesT")
        edgesT = singles.tile([128, 2, E], F16, tag="edgesT")
        ownT = singles.tile([128, 2, R], F16, tag="ownT")
        WN = singles.tile([128, 2, H * D], F16, tag="WN")
        WE = singles.tile([128, 2, H * D], F16, tag="WE")
        WNT = singles.tile([D, H, F_IN], F16, tag="WNT")
        WET = singles.tile([D, H, F_IN], F16, tag="WET")
        aN = singles.tile([D, 3 * H], F16, tag="aN")
        aE = singles.tile([D, H], F16, tag="aE")
        # per-j exponential vectors (j-chunk partition layout)
        e10n = singles.tile([128, NCH, 3 * H], F32, tag="e10n")  # e^{tgt} cols
        e2n = singles.tile([128, NCH, 3 * H], F32, tag="e2n")    # e^{0.2 tgt}
        e10e = singles.tile([128, ECH, H], F32, tag="e10e")
        e2e = singles.tile([128, ECH, H], F32, tag="e2e")
        # V tiles: [ht | 1] per (j-chunk, head); 66-wide for 4B alignment
        VT = singles.tile([128, NCH + ECH, H, 66], F16, tag="VT")
        # P' broadcast tiles per (head, part): [j-partition-bcast, 256 i]
        Pb = singles.tile([128, H, 2, R], F16, tag="Pb")
        ptsb = singles.tile([2 * H, RC, 128], F16, tag="ptsb")
        sel16 = singles.tile([2 * H, 2 * H * 128], F16, tag="sel16")
        identF16 = singles.tile([128, 128], F16, tag="identF16")
        identF32 = singles.tile([128, 128], F32, tag="identF32")
        acc = singles.tile([128, RC, D], F32, tag="acc")

        make_identity(nc, identF16)
        make_identity(nc, identF32)
        nc.gpsimd.memset(sel16, 0.0)
        nc.gpsimd.affine_select(
            out=sel16, in_=sel16, compare_op=mybir.AluOpType.not_equal,
            fill=1.0, base=0, pattern=[[-1, 2 * H], [0, 128]],
            channel_multiplier=1,
        )
        nc.vector.memset(VT[:, :, :, 64:66], 0.0)
        nc.vector.memset(VT[:, :, :, 64:65], 1.0)
        nc.vector.memset(acc, 0.0)

        # ---- input DMAs (critical-path tensors first; masks split so the
        # first j-chunks land early) ---------------------------------------
        nc.sync.dma_start(out=aN, in_=aN_d[:, :])
        nc.sync.dma_start(out=aE, in_=aE_d[:, :])
        nc.sync.dma_start(out=WNT, in_=WNT_d.rearrange("d (h k) -> d h k", h=H))
        nc.sync.dma_start(out=WET, in_=WET_d.rearrange("d (h k) -> d h k", h=H))
        nc.sync.dma_start(out=ownT, in_=ownT_d.rearrange("(c p) n -> p c n", p=128))
        nc.sync.dma_start(out=nodesT, in_=nodesT_d.rearrange("(c p) n -> p c n", p=128))
        nc.sync.dma_start(out=WN, in_=WN_d.rearrange("(c p) m -> p c m", p=128))
        nc.sync.dma_start(out=maskN, in_=maskN_d.rearrange("(t p) i -> p t i", p=128))
        nc.sync.dma_start(out=edgesT, in_=edgesT_d.rearrange("(c p) n -> p c n", p=128))
        nc.sync.dma_start(out=WE, in_=WE_d.rearrange("(c p) m -> p c m", p=128))
        nc.sync.dma_start(out=maskE, in_=maskE_d.rearrange("(t p) i -> p t i", p=128))

        # ---- attention weight vectors: wv[k, v] = sum_d W[k, hd] a[h, d] --
        wvN = singles.tile([128, 2, 3 * H], F16, tag="wvN")
        wvE = singles.tile([128, 2, H], F16, tag="wvE")
        Copy = mybir.ActivationFunctionType.Copy
        Exp = mybir.ActivationFunctionType.Exp
        pwv = psum_misc.tile([128, 2, 3 * H], F32, tag="pm")
        pwe = psum_misc.tile([128, 2, H], F32, tag="pm")
        for kc in range(2):
            for h in range(H):
                nc.tensor.matmul(pwv[:, kc, 3 * h:3 * h + 3],
                                 WNT[:, h, kc * 128:(kc + 1) * 128],
                                 aN[:, 3 * h:3 * h + 3])
                nc.tensor.matmul(pwe[:, kc, h:h + 1],
                                 WET[:, h, kc * 128:(kc + 1) * 128],
                                 aE[:, h:h + 1])
        nc.scalar.activation(wvN[:, :, :], pwv[:, :, :], Copy)
        nc.scalar.activation(wvE[:, :, :], pwe[:, :, :], Copy)

        # ---- own-row P' = e^{0.8 src} -> free-dim broadcast tiles (first:
        # everything the main loop's first iterations depend on) -----------
        e8own = small.tile([128, RC, 3 * H], F16, tag="e8own")
        pso = psum_misc.tile([128, RC, 3 * H], F32, tag="pm")
        for ch in range(RC):
            for kc in range(2):
                nc.tensor.matmul(pso[:, ch, :],
                                 ownT[:, kc, ch * 128:(ch + 1) * 128],
                                 wvN[:, kc, :], start=(kc == 0), stop=(kc == 1))
        nc.scalar.activation(e8own[:, :, :], pso[:, :, :], Exp, scale=0.8)
        # transpose the 16 needed src columns (u = 2h+part <- col 3h+2*part)
        # to [16, i] rows at base partition 0, then broadcast each row across
        # all 128 partitions with a selector matmul (row u of sel16 block u
        # is all-ones).
        e8cols = small.tile([128, RC, 2 * H], F16, tag="e8cols")
        for ch in range(RC):
            e8sl = e8own[:, ch, :]
            cols = bass.AP(tensor=e8sl.tensor, offset=e8sl.offset,
                           ap=[e8sl.ap[0], [3, H], [2, 2]])
            nc.scalar.activation(e8cols[:, ch, :], cols, Copy)
            pt = psum_misc.tile([2 * H, 128], F16, tag="pm")
            nc.tensor.transpose(pt[:, :], e8cols[:, ch, :], identF16[:, :])
            nc.scalar.activation(ptsb[:, ch, :], pt[:, :], Copy)
        for u in range(2 * H):
            h, part = u // 2, u % 2
            pb = psum_misc.tile([128, R], F32, tag="pm")
            nc.tensor.matmul(pb[:, :], sel16[:, u * 128:(u + 1) * 128],
                             ptsb[:, :, :].rearrange("u c p -> u (c p)"))
            nc.vector.tensor_copy(Pb[:, h, part, :], pb[:, :])

        # ---- src/tgt scores -> per-j exponentials (batched exps) ---------
        psn = psum_misc.tile([128, NCH, 3 * H], F32, tag="pm")
        for ch in range(NCH):
            for kc in range(2):
                nc.tensor.matmul(psn[:, ch, :],
                                 nodesT[:, kc, ch * 128:(ch + 1) * 128],
                                 wvN[:, kc, :], start=(kc == 0), stop=(kc == 1))
        nc.scalar.activation(e10n[:, :, :], psn[:, :, :], Exp, scale=1.0)
        nc.scalar.activation(e2n[:, :, :], psn[:, :, :], Exp, scale=0.2)
        # ---- ht = emb @ W, stored as [ht | 1] fp16 V tiles ---------------
        def emit_ht(ch):
            ph = psum_ht.tile([128, H * D], F32, tag="ph")
            for kc in range(2):
                if ch < NCH:
                    lhsT = nodesT[:, kc, ch * 128:(ch + 1) * 128]
                    rhs = WN[:, kc, :]
                else:
                    lhsT = edgesT[:, kc, (ch - NCH) * 128:(ch - NCH + 1) * 128]
                    rhs = WE[:, kc, :]
                nc.tensor.matmul(ph[:, :], lhsT, rhs, start=(kc == 0), stop=(kc == 1))
            nc.scalar.activation(
                VT[:, ch, :, 0:64],
                ph[:, :].rearrange("p (h d) -> p h d", h=H),
                Copy,
            )

        for ch in range(NCH):
            emit_ht(ch)

        def emit_edges_prep():
            pse = psum_misc.tile([128, ECH, H], F32, tag="pm")
            for ch in range(ECH):
                for kc in range(2):
                    nc.tensor.matmul(pse[:, ch, :],
                                     edgesT[:, kc, ch * 128:(ch + 1) * 128],
                                     wvE[:, kc, :], start=(kc == 0), stop=(kc == 1))
            nc.scalar.activation(e10e[:, :, :], pse[:, :, :], Exp, scale=1.0)
            nc.scalar.activation(e2e[:, :, :], pse[:, :, :], Exp, scale=0.2)
            for ch in range(NCH, NCH + ECH):
                emit_ht(ch)

        # ---- main loop: u = mask * max(P'*QT, T); S += u^T-contract V ----
        # all N-parts first (node-side inputs arrive early), then all
        # E-parts - the edge-side DMAs hide under the N-part compute.
        G = 8  # j-chunks per fused mask-multiply
        for part in range(2):
            njt = NCH if part == 0 else ECH
            for h in range(H):
                if part == 0 and h == 3:
                    emit_edges_prep()
                Sp = psum_acc.tile([65, R], F32, tag="Sacc")
                for jt0 in range(0, njt, G):
                    dt_ = work.tile([128, G, R], F16, tag="Dt")
                    for g in range(G):
                        jt = jt0 + g
                        if part == 0:
                            q10 = e10n[:, jt, 3 * h + 1:3 * h + 2]
                            q2 = e2n[:, jt, 3 * h + 1:3 * h + 2]
                        else:
                            q10 = e10e[:, jt, h:h + 1]
                            q2 = e2e[:, jt, h:h + 1]
                        nc.vector.tensor_scalar(
                            out=dt_[:, g, :], in0=Pb[:, h, part, :],
                            scalar1=q10, scalar2=q2,
                            op0=mybir.AluOpType.mult, op1=mybir.AluOpType.max,
                        )
                    ut = work.tile([128, G, R], F16, tag="ut")
                    msk = maskN if part == 0 else maskE
                    nc.vector.tensor_mul(ut[:, :, :], dt_[:, :, :],
                                         msk[:, jt0:jt0 + G, :])
                    for g in range(G):
                        jt = jt0 + g
                        vch = jt if part == 0 else NCH + jt
                        nc.tensor.matmul(
                            Sp[:, :], VT[:, vch, h, 0:65], ut[:, g, :],
                            start=(jt == 0), stop=(jt == njt - 1),
                        )
                # ---- normalize + accumulate into acc -------------------
                Ssb = small.tile([65, R], F32, tag="Ssb")
                nc.scalar.activation(Ssb[:, :], Sp[:, :], Copy)
                for ch in range(RC):
                    ptt = psum_misc.tile([128, 65], F32, tag="pm")
                    nc.tensor.transpose(ptt[:, :], Ssb[:, ch * 128:(ch + 1) * 128],
                                        identF32[:65, :65])
                    Tt = small.tile([128, 65], F32, tag="Tt")
                    nc.scalar.activation(Tt[:, :], ptt[:, :], Copy)
                    rec = small.tile([128, 1], F32, tag="rec")
                    nc.vector.reciprocal(rec[:, :], Tt[:, 64:65])
                    contrib = small.tile([128, D], F32, tag="contrib")
                    nc.scalar.activation(contrib[:, :], Tt[:, 0:64], Copy,
                                         scale=rec[:, :])
                    nc.vector.tensor_add(acc[:, ch, :], acc[:, ch, :], contrib[:, :])

        out_sb = singles.tile([128, RC, D], F32, tag="out_sb")
        nc.scalar.mul(out_sb[:, :, :], acc[:, :, :], 1.0 / H)
        nc.sync.dma_start(out=out_d.rearrange("(c p) d -> p c d", p=128), in_=out_sb)

    return nc


def _get_program() -> bass.Bass:
    global _PROGRAM
    if _PROGRAM is None:
        nc = _build_program()
        nc.finalize()
        _PROGRAM = nc
    return _PROGRAM


def _prepare_in_maps(inputs) -> list:
    nodes = np.ascontiguousarray(np.asarray(inputs["nodes_embeddings"], np.float32))
    edges = np.ascontiguousarray(np.asarray(inputs["edges_embeddings"], np.float32))
    WN = np.asarray(inputs["WN"], np.float32)
    WE = np.asarray(inputs["WE"], np.float32)
    aN = np.asarray(inputs["aN"], np.float32)
    aE = np.asarray(inputs["aE"], np.float32)
    mat_nodes = np.asarray(inputs["mat_nodes"])
    mat_edges = np.asarray(inputs["mat_edges"])

    f16 = np.float16
    nodesT16 = np.ascontiguousarray(nodes.T.astype(f16))
    edgesT16 = np.ascontiguousarray(edges.T.astype(f16))
    WN16 = WN.astype(f16)
    WE16 = WE.astype(f16)
    WNT16 = np.ascontiguousarray(
        WN.T.astype(f16).reshape(H, D, F_IN).transpose(1, 0, 2).reshape(D, H * F_IN))
    WET16 = np.ascontiguousarray(
        WE.T.astype(f16).reshape(H, D, F_IN).transpose(1, 0, 2).reshape(D, H * F_IN))
    # aN16 cols per head: [srcN, tgtN, srcE]; aE16 col per head: tgtE
    aN16 = np.empty((D, 3 * H), f16)
    aE16 = np.empty((D, H), f16)
    for h in range(H):
        aN16[:, 3 * h] = aN[h, :D, 0].astype(f16)
        aN16[:, 3 * h + 1] = aN[h, D:, 0].astype(f16)
        aN16[:, 3 * h + 2] = aE[h, :D, 0].astype(f16)
        aE16[:, h] = aE[h, D:, 0].astype(f16)
    maskN_T = np.ascontiguousarray(mat_nodes.astype(f16).T)  # [j, i_global]
    maskE_T = np.ascontiguousarray(mat_edges.astype(f16).T)

    in_maps = []
    for c in range(NCORES):
        sl = slice(c * R, (c + 1) * R)
        in_maps.append({
            "maskN": np.ascontiguousarray(maskN_T[:, sl]),
            "maskE": np.ascontiguousarray(maskE_T[:, sl]),
            "nodesT": nodesT16,
            "edgesT": edgesT16,
            "ownT": np.ascontiguousarray(nodesT16[:, sl]),
            "WN16": WN16,
            "WE16": WE16,
            "WNT16": WNT16,
            "WET16": WET16,
            "aN16": aN16,
            "aE16": aE16,
        })
    return in_maps


def kernel(**inputs) -> np.ndarray:
    in_maps = _prepare_in_maps(inputs)
    nc = _get_program()
    res = run_bass_kernel_spmd(nc, in_maps, core_ids=list(range(NCORES)))
    return np.concatenate([res.results[c]["out"] for c in range(NCORES)], axis=0)
